# revision 3
# baseline (speedup 1.0000x reference)
"""GNN edge-softmax attention kernel for 8 Trainium2 NeuronCores.

Strategy (4 src-rows x 2 dst-halves core grid):
  - Host routes each edge to core (row(src), half(dst)). Nodes are packed
    into 128-node tiles balanced by edge count; each tile's edges are padded
    to whole 128-edge blocks so every core runs an identical program.
  - Per core: project the FULL dst-half k/v into an f16 khv table in HBM
    (no collective), project q into an SBUF table, then for each node tile:
    gather khv[dst] rows with dma_gather, compute per-edge logits (DVE mult
    + grouped reduce, PE for the edge-feature term), exp on ACT, and
    scatter-accumulate numerator/denominator into PSUM with one-hot matmuls.
  - num/den partials are AllReduced pairwise (f16, two chunks, issued after
    the last gather so the collective never blocks the gather chain), then
    each core normalizes and applies the output projection.
"""

import math
import sys

import numpy as np

sys.path.insert(0, "/opt/trn_rl_repo")

import concourse.bacc as bacc
import concourse.bass as bass
import concourse.mybir as mybir
import concourse.tile as tile
from concourse import bass_utils

F16 = mybir.dt.float16
F8 = mybir.dt.float8e4
F32 = mybir.dt.float32
I16 = mybir.dt.int16

H = 8            # heads
D = 16           # head dim
TD = H * D       # 128
QD = 256         # q/k/v feature dim
PD = 64          # edge pair feature dim
R = 4            # src rows of the core grid
C = 2            # dst cols of the core grid
P = 128

AF = mybir.ActivationFunctionType
ALU = mybir.AluOpType


def _wrap16(idx: np.ndarray) -> np.ndarray:
    """dma_gather index layout: [128, n/16] with idx i at (i%16 + 16k, i//16)."""
    n = idx.shape[0]
    assert n % 16 == 0
    w = idx.reshape(n // 16, 16).T.astype(np.int16)  # [16, n/16]
    return np.tile(w, (8, 1))  # replicate across the 8 partition groups


def prepare(q, k, v, edges, edge_index, Wq, Wk, Wv, Wb, bb, Wo, bo):
    N = q.shape[0]
    E = edges.shape[0]
    ntiles_row = math.ceil(N / (R * P))          # tiles per src row
    NROW = ntiles_row * P                        # nodes per row (padded)
    NPAD = NROW * R
    DHALF = NPAD // 2                            # dst-half size
    assert DHALF < 32768, "dst half must fit int16"

    src = np.asarray(edge_index[:, 0], dtype=np.int64)
    dst = np.asarray(edge_index[:, 1], dtype=np.int64)
    deg = np.bincount(src, minlength=N)

    # --- greedy node->tile packing balanced by edge count ---
    T = R * ntiles_row
    order = np.argsort(-deg, kind="stable")
    tile_cnt = np.zeros(T, dtype=np.int64)       # nodes in tile
    tile_edges = np.zeros(T, dtype=np.int64)
    node_tile = np.zeros(N, dtype=np.int32)
    node_slot = np.zeros(N, dtype=np.int32)
    import heapq
    heap = [(0, t) for t in range(T)]
    heapq.heapify(heap)
    for n in order:
        while True:
            e_cnt, t = heapq.heappop(heap)
            if tile_cnt[t] < P:
                break
        node_tile[n] = t
        node_slot[n] = tile_cnt[t]
        tile_cnt[t] += 1
        tile_edges[t] += deg[n]
        if tile_cnt[t] < P:
            heapq.heappush(heap, (tile_edges[t], t))

    # --- align heavy tiles across rows to shrink shared block counts ---
    # bpt is shared across all 8 cores, so sort each row's tiles by their own
    # per-core max edge count; slot s then holds every row's s-th heaviest
    # tile and sum_s max_cores cnt shrinks vs arbitrary alignment.
    j_of_edge0 = (dst // DHALF).astype(np.int64)
    cnt_rc = np.zeros((R, C, ntiles_row), dtype=np.int64)
    np.add.at(cnt_rc,
              (node_tile[src] // ntiles_row, j_of_edge0,
               node_tile[src] % ntiles_row), 1)
    perm = np.zeros((R, ntiles_row), dtype=np.int64)  # new tloc -> old tloc
    for r in range(R):
        key = cnt_rc[r].max(axis=0)
        perm[r] = np.argsort(-key, kind="stable")
    inv_perm = np.zeros_like(perm)
    for r in range(R):
        inv_perm[r, perm[r]] = np.arange(ntiles_row)
    old_row = node_tile // ntiles_row
    node_tile = (old_row * ntiles_row +
                 inv_perm[old_row, node_tile % ntiles_row]).astype(np.int32)

    row_of_edge = node_tile[src] // ntiles_row
    j_of_edge = (dst // DHALF).astype(np.int64)
    tloc_of_edge = (node_tile[src] % ntiles_row).astype(np.int64)

    # per (core, tile_local) edge counts -> shared block counts per tile slot
    core_of_edge = row_of_edge * C + j_of_edge
    cnt = np.zeros((R * C, ntiles_row), dtype=np.int64)
    np.add.at(cnt, (core_of_edge, tloc_of_edge), 1)
    bpt = np.maximum(1, np.ceil(cnt.max(axis=0) / P).astype(np.int64))  # [ntiles_row]
    blk_off = np.concatenate([[0], np.cumsum(bpt)])   # block offset per tile
    NBLK = int(blk_off[-1])
    ECAP = NBLK * P

    # --- per-core edge arrays ---
    cores = []
    eT_all = np.asarray(edges, dtype=np.float32).T    # [PD, E]
    import ml_dtypes
    F8NP = ml_dtypes.float8_e4m3
    for core in range(R * C):
        i, j = core // C, core % C
        mask = core_of_edge == core
        es, ed, et = src[mask], dst[mask], tloc_of_edge[mask]
        # order edges by tile slot
        ordr = np.argsort(et, kind="stable")
        es, ed, et = es[ordr], ed[ordr], et[ordr]
        # positions: per tile, fill from blk_off[t]*P
        pos = np.zeros(len(es), dtype=np.int64)
        start = 0
        for t in range(ntiles_row):
            c = int((et == t).sum())
            pos[start:start + c] = blk_off[t] * P + np.arange(c)
            start += c
        eidx = np.nonzero(mask)[0][ordr]

        dst_local = np.zeros(ECAP, dtype=np.int16)
        src_rel = np.full(ECAP, 255, dtype=np.int64)
        edgesT = np.zeros((PD + 1, ECAP), dtype=np.float16)
        edgesT[PD, :] = 1.0
        dst_local[pos] = (ed - j * DHALF).astype(np.int16)
        src_rel[pos] = node_slot[es]
        edgesT[:PD, pos] = eT_all[:, eidx].astype(np.float16)
        # one-hot selection matrices (fp8, exact 0/1)
        S_en = np.zeros((ECAP, P), dtype=F8NP)
        valid = src_rel < P
        S_en[np.nonzero(valid)[0], src_rel[valid]] = 1.0
        S_en3 = S_en.reshape(NBLK, P, P)                       # [b, e, n]
        S_mat = np.ascontiguousarray(S_en3.transpose(1, 0, 2)).reshape(P, ECAP)   # [e_part, (b n)]
        ST_mat = np.ascontiguousarray(S_en3.transpose(2, 0, 1)).reshape(P, ECAP)  # [n_part, (b e)]

        # constants: this core projects the FULL khv table of its half j
        qlo = j * DHALF
        qhi = min(qlo + DHALF, N)
        kT = np.zeros((QD, DHALF), dtype=np.float16)
        vT = np.zeros((QD, DHALF), dtype=np.float16)
        kT[:, :qhi - qlo] = np.asarray(k[qlo:qhi], np.float32).T.astype(np.float16)
        vT[:, :qhi - qlo] = np.asarray(v[qlo:qhi], np.float32).T.astype(np.float16)
        # q rows permuted into (tile_local, slot) order for this row i
        qT = np.zeros((QD, NROW), dtype=np.float16)
        rmask = node_tile // ntiles_row == i
        rn = np.nonzero(rmask)[0]
        qpos = (node_tile[rn] % ntiles_row) * P + node_slot[rn]
        qT[:, qpos] = np.asarray(q[rn], np.float32).T.astype(np.float16)

        cores.append(dict(
            dst_idx=_wrap16(dst_local), S_mat=S_mat, ST_mat=ST_mat,
            edgesT=edgesT, kT=kT, vT=vT, qT=qT,
        ))

    norm = D ** -0.5
    consts = dict(
        WkT=np.asarray(Wk, np.float32).T.astype(np.float16),
        WvT=np.asarray(Wv, np.float32).T.astype(np.float16),
        WqT=(np.asarray(Wq, np.float32) * norm).T.astype(np.float16),
        WbT_aug=np.concatenate(
            [np.asarray(Wb, np.float32).T,
             np.asarray(bb, np.float32)[None, :]], axis=0).astype(np.float16),
        WoT=np.asarray(Wo, np.float32).T.astype(np.float16),
        bo_row=np.asarray(bo, np.float32)[None, :].astype(np.float16),
        identity=np.eye(P, dtype=np.float16),
        ones_col=np.ones((1, P), dtype=np.float16),
    )
    meta = dict(N=N, NPAD=NPAD, NROW=NROW, DHALF=DHALF, ntiles_row=ntiles_row,
                NBLK=NBLK, ECAP=ECAP, bpt=bpt.tolist(), blk_off=blk_off.tolist(),
                node_tile=node_tile, node_slot=node_slot, deg=deg)
    return cores, consts, meta


def build_program(meta, gather_batch=3):
    """Build the SPMD bass program. Returns compiled nc."""
    ntr = meta["ntiles_row"]
    NROW, DHALF = meta["NROW"], meta["DHALF"]
    NBLK, ECAP = meta["NBLK"], meta["ECAP"]
    bpt, blk_off = meta["bpt"], meta["blk_off"]
    NKV = DHALF // P      # khv chunks (full half)
    NQ = NROW // P        # qh chunks
    H1 = (ntr + 1) // 2   # tiles in the first reduce/finalize chunk

    nc = bacc.Bacc("TRN2", target_bir_lowering=False, debug=False, num_devices=R * C)
    dt = nc.dram_tensor
    # inputs
    t_dst = dt("dst_idx", [P, ECAP // 16], I16, kind="ExternalInput").ap()
    t_S = dt("S_mat", [P, ECAP], F8, kind="ExternalInput").ap()
    t_ST = dt("ST_mat", [P, ECAP], F8, kind="ExternalInput").ap()
    t_eT = dt("edgesT", [PD + 1, ECAP], F16, kind="ExternalInput").ap()
    t_kT = dt("kT", [QD, DHALF], F16, kind="ExternalInput").ap()
    t_vT = dt("vT", [QD, DHALF], F16, kind="ExternalInput").ap()
    t_qT = dt("qT", [QD, NROW], F16, kind="ExternalInput").ap()
    t_WkT = dt("WkT", [QD, TD], F16, kind="ExternalInput").ap()
    t_WvT = dt("WvT", [QD, TD], F16, kind="ExternalInput").ap()
    t_WqT = dt("WqT", [QD, TD], F16, kind="ExternalInput").ap()
    t_Wb = dt("WbT_aug", [PD + 1, H], F16, kind="ExternalInput").ap()
    t_WoT = dt("WoT", [TD, QD], F16, kind="ExternalInput").ap()
    t_bo = dt("bo_row", [1, QD], F16, kind="ExternalInput").ap()
    t_id = dt("identity", [P, P], F16, kind="ExternalInput").ap()
    t_ones = dt("ones_col", [1, P], F16, kind="ExternalInput").ap()
    # internal DRAM
    t_khv = dt("khv_tab", [DHALF, 2 * TD], F16).ap()
    t_nd_a = dt("nd_part_a", [H1 * P, 136], F16).ap()
    t_ndr_a = dt("nd_red_a", [H1 * P, 136], F16).ap()
    t_nd_b = dt("nd_part_b", [(ntr - H1) * P, 136], F16).ap()
    t_ndr_b = dt("nd_red_b", [(ntr - H1) * P, 136], F16).ap()
    # output (every core finalizes all tiles of its row; host reads j=0 cores)
    t_out = dt("o_out", [ntr * P, QD], F16, kind="ExternalOutput").ap()

    GB = gather_batch

    with tile.TileContext(nc) as tc:
        with (
            tc.tile_pool(name="const", bufs=1) as cpool,
            tc.tile_pool(name="proj", bufs=3) as ppool,
            tc.tile_pool(name="gath", bufs=3) as gpool,
            tc.tile_pool(name="work", bufs=2) as wpool,
            tc.tile_pool(name="out", bufs=2) as opool,
            tc.tile_pool(name="psA", bufs=2, space="PSUM") as psA,
            tc.tile_pool(name="psB", bufs=2, space="PSUM") as psB,
            tc.tile_pool(name="psC", bufs=1, space="PSUM") as psC,
        ):
            # ---- constants to SBUF ----
            c_WkT = cpool.tile([P, 2 * TD], F16)
            nc.sync.dma_start(out=c_WkT[:, 0:TD], in_=t_WkT[0:P, :])
            nc.sync.dma_start(out=c_WkT[:, TD:2 * TD], in_=t_WkT[P:QD, :])
            c_WvT = cpool.tile([P, 2 * TD], F16)
            nc.sync.dma_start(out=c_WvT[:, 0:TD], in_=t_WvT[0:P, :])
            nc.sync.dma_start(out=c_WvT[:, TD:2 * TD], in_=t_WvT[P:QD, :])
            c_WqT = cpool.tile([P, 2 * TD], F16)
            nc.sync.dma_start(out=c_WqT[:, 0:TD], in_=t_WqT[0:P, :])
            nc.sync.dma_start(out=c_WqT[:, TD:2 * TD], in_=t_WqT[P:QD, :])
            c_Wb = cpool.tile([PD + 1, H], F16); nc.sync.dma_start(out=c_Wb[:], in_=t_Wb)
            c_WoT = cpool.tile([TD, QD], F16); nc.sync.dma_start(out=c_WoT[:], in_=t_WoT)
            c_bo = cpool.tile([1, QD], F16); nc.sync.dma_start(out=c_bo[:], in_=t_bo)
            c_id = cpool.tile([P, P], F16); nc.sync.dma_start(out=c_id[:], in_=t_id)
            c_ones = cpool.tile([1, P], F16); nc.sync.dma_start(out=c_ones[:], in_=t_ones)
            c_dsti = cpool.tile([P, ECAP // 16], I16)
            nc.sync.dma_start(out=c_dsti[:], in_=t_dst)
            qh_sb = cpool.tile([P, NQ * TD], F16)

            # ---- phase A: projections (khv -> HBM table, qh -> SBUF) ----
            def project_khv():
                for g0 in range(0, NKV, 8):
                    g1 = min(g0 + 8, NKV)
                    w = (g1 - g0) * P
                    ka = ppool.tile([P, 1024], F16, tag="ka")
                    kb = ppool.tile([P, 1024], F16, tag="kb")
                    va = ppool.tile([P, 1024], F16, tag="va")
                    vb = ppool.tile([P, 1024], F16, tag="vb")
                    nc.sync.dma_start(out=ka[:, :w], in_=t_kT[0:P, g0 * P:g0 * P + w])
                    nc.sync.dma_start(out=kb[:, :w], in_=t_kT[P:QD, g0 * P:g0 * P + w])
                    nc.sync.dma_start(out=va[:, :w], in_=t_vT[0:P, g0 * P:g0 * P + w])
                    nc.sync.dma_start(out=vb[:, :w], in_=t_vT[P:QD, g0 * P:g0 * P + w])
                    stage = ppool.tile([P, 8 * 2 * TD], F16, tag="kvstage")
                    for cc in range(g0, g1):
                        o = (cc - g0) * P
                        psq = psA.tile([P, 4 * TD], F32, tag="qsel")
                        ps = psq[:, 0:TD]
                        nc.tensor.matmul(out=ps, lhsT=ka[:, o:o + P], rhs=c_WkT[:, 0:TD],
                                         start=True, stop=False)
                        nc.tensor.matmul(out=ps, lhsT=kb[:, o:o + P], rhs=c_WkT[:, TD:2 * TD],
                                         start=False, stop=True)
                        nc.scalar.activation(out=stage[:, (cc - g0) * 256:(cc - g0) * 256 + TD],
                                             in_=ps, func=AF.Copy)
                        ps2 = psq[:, TD:2 * TD]
                        nc.tensor.matmul(out=ps2, lhsT=va[:, o:o + P], rhs=c_WvT[:, 0:TD],
                                         start=True, stop=False)
                        nc.tensor.matmul(out=ps2, lhsT=vb[:, o:o + P], rhs=c_WvT[:, TD:2 * TD],
                                         start=False, stop=True)
                        nc.scalar.activation(
                            out=stage[:, (cc - g0) * 256 + TD:(cc - g0) * 256 + 2 * TD],
                            in_=ps2, func=AF.Copy)
                    nc.sync.dma_start(
                        out=t_khv[g0 * P:g1 * P, :].rearrange("(c p) w -> p c w", p=P),
                        in_=stage[:, :(g1 - g0) * 256].rearrange(
                            "p (c w) -> p c w", w=256))

            def project_qh():
                for g0 in range(0, NQ, 8):
                    g1 = min(g0 + 8, NQ)
                    w = (g1 - g0) * P
                    ina = ppool.tile([P, 1024], F16, tag="ina")
                    inb = ppool.tile([P, 1024], F16, tag="inb")
                    nc.sync.dma_start(out=ina[:, :w], in_=t_qT[0:P, g0 * P:g0 * P + w])
                    nc.sync.dma_start(out=inb[:, :w], in_=t_qT[P:QD, g0 * P:g0 * P + w])
                    for cc in range(g0, g1):
                        o = (cc - g0) * P
                        psq = psA.tile([P, 4 * TD], F32, tag="qsel")
                        ps = psq[:, 0:TD]
                        nc.tensor.matmul(out=ps, lhsT=ina[:, o:o + P], rhs=c_WqT[:, 0:TD],
                                         start=True, stop=False)
                        nc.tensor.matmul(out=ps, lhsT=inb[:, o:o + P], rhs=c_WqT[:, TD:2 * TD],
                                         start=False, stop=True)
                        nc.scalar.activation(out=qh_sb[:, cc * TD:(cc + 1) * TD],
                                             in_=ps, func=AF.Copy)

            project_khv()
            project_qh()

            def reduce_chunk(t_in, t_out):
                nc.gpsimd.collective_compute(
                    "AllReduce", ALU.add,
                    replica_groups=[[0, 1], [2, 3], [4, 5], [6, 7]],
                    ins=[t_in], outs=[t_out])

            # ---- finalize: normalize + output projection for one tile ----
            def finalize(t):
                src_nd = t_ndr_a if t < H1 else t_ndr_b
                tt = t if t < H1 else t - H1
                ndl = opool.tile([P, 136], F16, tag="ndl")
                nc.sync.dma_start(out=ndl[:], in_=src_nd[tt * P:(tt + 1) * P, :])
                rden = opool.tile([P, H], F32, tag="rden")
                # +eps so empty node slots yield 0 instead of 0*inf=NaN
                nc.vector.tensor_scalar_add(out=rden[:], in0=ndl[:, TD:TD + H],
                                            scalar1=1e-30)
                nc.vector.reciprocal(out=rden[:], in_=rden[:])
                o_sb = opool.tile([P, TD], F16, tag="o_sb")
                nc.vector.tensor_tensor(
                    out=o_sb[:].rearrange("p (h d) -> p h d", h=H),
                    in0=ndl[:, 0:TD].rearrange("p (h d) -> p h d", h=H),
                    in1=rden[:, :, None].to_broadcast([P, H, D]),
                    op=ALU.mult)
                ps_oT = psC.tile([P, P], F16, tag="oT")
                nc.tensor.transpose(out=ps_oT[:], in_=o_sb[:], identity=c_id[:])
                oT_sb = opool.tile([P, P], F16, tag="oT_sb")
                nc.scalar.activation(out=oT_sb[:], in_=ps_oT[:], func=AF.Copy)
                ps_o = psC.tile([P, QD], F32, tag="ps_o")
                nc.tensor.matmul(out=ps_o[:], lhsT=oT_sb[:], rhs=c_WoT[:],
                                 start=True, stop=False)
                nc.tensor.matmul(out=ps_o[:], lhsT=c_ones[:], rhs=c_bo[:],
                                 start=False, stop=True)
                out_sb = opool.tile([P, QD], F16, tag="out_sb")
                nc.scalar.activation(out=out_sb[:], in_=ps_o[:], func=AF.Copy)
                nc.sync.dma_start(out=t_out[t * P:(t + 1) * P, :], in_=out_sb[:])

            # ---- phase M: main loop over node tiles ----
            batches = []
            t0 = 0
            while t0 < ntr:
                t1 = min(t0 + GB, ntr)
                batches.append((t0, t1))
                t0 = t1

            def compute_tile(t, e0, khv_g, eT, S_sb, ST_sb):
                nb = bpt[t]
                go = blk_off[t] * P - e0     # edge offset in gather batch
                gb = go // P                 # block offset in gather batch
                # qh rows via one-hot matmul, 4 blocks per PSUM bank;
                # qk product reads PSUM directly
                prod = wpool.tile([P, nb, H, D], F16, tag="prod")
                qk = wpool.tile([P, nb, H], F16, tag="qk")
                for c0 in range(0, nb, 4):
                    ch = min(4, nb - c0)
                    ps_q = psA.tile([P, 4 * TD], F32, tag="qsel")
                    for b in range(c0, c0 + ch):
                        nc.tensor.matmul(out=ps_q[:, (b - c0) * TD:(b - c0 + 1) * TD],
                                         lhsT=ST_sb[:, go + b * P:go + (b + 1) * P],
                                         rhs=qh_sb[:, t * TD:(t + 1) * TD],
                                         start=True, stop=True)
                    nc.vector.tensor_tensor(
                        out=prod[:, c0:c0 + ch, :, :],
                        in0=ps_q[:, :ch * TD].rearrange("p (c h d) -> p c h d", c=ch, h=H),
                        in1=khv_g[:, gb + c0:gb + c0 + ch, 0:TD].rearrange(
                            "p c (h d) -> p c h d", h=H),
                        op=ALU.mult)
                    with nc.allow_low_precision(reason="f16 qk logits are within tolerance"):
                        nc.vector.reduce_sum(out=qk[:, c0:c0 + ch, :],
                                             in_=prod[:, c0:c0 + ch, :, :],
                                             axis=mybir.AxisListType.X)
                # edge-feature logits on PE: eb[e, h] per block
                ps_eb = psB.tile([P, nb * H], F32, tag="eb")
                for b in range(nb):
                    nc.tensor.matmul(out=ps_eb[:, b * H:(b + 1) * H],
                                     lhsT=eT[:, go + b * P:go + (b + 1) * P],
                                     rhs=c_Wb[:], start=True, stop=True)
                attn = wpool.tile([P, nb * H], F32, tag="attn")
                nc.vector.tensor_tensor(out=attn[:], in0=qk[:].rearrange("p b h -> p (b h)"),
                                        in1=ps_eb[:], op=ALU.add)
                w_t = wpool.tile([P, nb, H], F16, tag="w")
                nc.scalar.activation(out=w_t[:].rearrange("p b h -> p (b h)"),
                                     in_=attn[:], func=AF.Exp)
                # rhs = [w*vh | w]
                wv = wpool.tile([P, nb, 136], F16, tag="wv")
                nc.vector.tensor_tensor(
                    out=wv[:, :, 0:TD].rearrange("p b (h d) -> p b h d", h=H),
                    in0=khv_g[:, gb:gb + nb, TD:2 * TD].rearrange("p b (h d) -> p b h d", h=H),
                    in1=w_t[:, :, :, None].to_broadcast([P, nb, H, D]),
                    op=ALU.mult)
                nc.vector.tensor_copy(out=wv[:, :, TD:TD + H], in_=w_t[:])
                # scatter-accumulate into num|den psum
                ps_nd = psB.tile([P, 136], F32, tag="nd")
                for b in range(nb):
                    nc.tensor.matmul(out=ps_nd[:],
                                     lhsT=S_sb[:, go + b * P:go + (b + 1) * P],
                                     rhs=wv[:, b, :],
                                     start=(b == 0), stop=(b == nb - 1))
                nd_sb = opool.tile([P, 136], F16, tag="nd_sb")
                nc.scalar.activation(out=nd_sb[:], in_=ps_nd[:], func=AF.Copy)
                if t < H1:
                    nc.sync.dma_start(out=t_nd_a[t * P:(t + 1) * P, :], in_=nd_sb[:])
                else:
                    nc.sync.dma_start(
                        out=t_nd_b[(t - H1) * P:(t - H1 + 1) * P, :], in_=nd_sb[:])

            # index of the batch holding the final gather: after emitting its
            # gather we can emit chunk-A's reduce without stalling the chain.
            last_bi = len(batches) - 1
            for bi, (b0, b1) in enumerate(batches):
                e0, e1 = blk_off[b0] * P, blk_off[b1] * P
                ne = e1 - e0
                khv_g = gpool.tile([P, ne // P, 2 * TD], F16, tag="khv_g")
                nc.gpsimd.dma_gather(
                    out_ap=khv_g[:], in_ap=t_khv,
                    idxs_ap=c_dsti[:, e0 // 16:e1 // 16],
                    num_idxs=ne, num_idxs_reg=ne, elem_size=2 * TD,
                    single_packet=False)
                if bi == last_bi:
                    # all gathers are in the gpsimd queue; the collective can
                    # follow them without blocking any gather.
                    reduce_chunk(t_nd_a, t_ndr_a)
                eT = gpool.tile([PD + 1, ne], F16, tag="eT")
                nc.sync.dma_start(out=eT[:], in_=t_eT[:, e0:e1])
                S_sb = gpool.tile([P, ne], F8, tag="S_sb")
                nc.sync.dma_start(out=S_sb[:], in_=t_S[:, e0:e1])
                ST_sb = gpool.tile([P, ne], F8, tag="ST_sb")
                nc.sync.dma_start(out=ST_sb[:], in_=t_ST[:, e0:e1])
                for t in range(b0, b1):
                    compute_tile(t, e0, khv_g, eT, S_sb, ST_sb)

            for tf in range(H1):
                finalize(tf)
            reduce_chunk(t_nd_b, t_ndr_b)
            for tf in range(H1, ntr):
                finalize(tf)

    nc.compile()
    return nc


_CACHE = {}
LAST_RUN = {}


def kernel(**inputs) -> np.ndarray:
    q = np.asarray(inputs["q"], np.float32)
    k = np.asarray(inputs["k"], np.float32)
    v = np.asarray(inputs["v"], np.float32)
    edges = np.asarray(inputs["edges"], np.float32)
    edge_index = np.asarray(inputs["edge_index"])
    Wq, Wk, Wv = inputs["Wq"], inputs["Wk"], inputs["Wv"]
    Wb, bb, Wo, bo = inputs["Wb"], inputs["bb"], inputs["Wo"], inputs["bo"]

    cores, consts, meta = prepare(q, k, v, edges, edge_index, Wq, Wk, Wv, Wb, bb, Wo, bo)
    N = meta["N"]
    ntr = meta["ntiles_row"]

    key = (q.shape, edges.shape, meta["NBLK"])
    if key not in _CACHE:
        _CACHE[key] = build_program(meta)
    nc = _CACHE[key]

    in_maps = []
    for core in range(R * C):
        m = dict(cores[core])
        m.update({kk: np.ascontiguousarray(vv) for kk, vv in consts.items()})
        in_maps.append({kk: np.ascontiguousarray(vv) for kk, vv in m.items()})

    import os
    if os.environ.get("KERNEL_SIM"):
        from concourse.bass_interp import MultiCoreSim
        sim = MultiCoreSim(nc, num_cores=R * C)
        for ci, core_sim in sim.cores.items():
            for name, arr in in_maps[ci].items():
                core_sim.tensor(name)[:] = arr
        sim.simulate(check_with_hw=False)
        results = [{"o_out": np.array(sim.cores[ci].tensor("o_out"))}
                   for ci in range(R * C)]
    else:
        trace = bool(os.environ.get("KERNEL_TRACE"))
        res = bass_utils.run_bass_kernel_spmd(nc, in_maps, core_ids=list(range(R * C)),
                                              trace=trace)
        LAST_RUN["res"] = res
        results = res.results

    # assemble: core (i, j=0) output has all ntr tiles of row i (both halves
    # reduced identically); use j=0 cores.
    out = np.zeros((meta["NPAD"], QD), np.float32)
    node_tile, node_slot = meta["node_tile"], meta["node_slot"]
    for i in range(R):
        o = results[i * C]["o_out"]  # [ntr*P, QD] f16
        out[i * ntr * P:(i + 1) * ntr * P] = np.asarray(o, np.float32)
    # map back to node ids
    full = np.zeros((N, QD), np.float32)
    rowpos = node_tile * P + node_slot
    full[:, :] = out[rowpos[np.arange(N)]]
    # zero-degree nodes: reference yields bo
    zd = meta["deg"] == 0
    if zd.any():
        full[zd] = np.asarray(bo, np.float32)[None, :]
    return full


# revision 5
# speedup vs baseline: 1.2335x; 1.2335x over previous
"""GNN edge-softmax attention kernel for 8 Trainium2 NeuronCores.

Strategy (4 src-rows x 2 dst-halves core grid):
  - Host routes each edge to core (row(src), half(dst)). Nodes are packed
    into 128-node tiles balanced by edge count; each tile's edges are padded
    to whole 128-edge blocks so every core runs an identical program.
  - Per core: project the FULL dst-half k/v into an f16 khv table in HBM
    (no collective), project q into an SBUF table, then for each node tile:
    gather khv[dst] rows with dma_gather, compute per-edge logits (DVE mult
    + grouped reduce, PE for the edge-feature term), exp on ACT, and
    scatter-accumulate numerator/denominator into PSUM with one-hot matmuls.
  - num/den partials are AllReduced pairwise (f16, two chunks, issued after
    the last gather so the collective never blocks the gather chain), then
    each core normalizes and applies the output projection.
"""

import math
import sys

import numpy as np

sys.path.insert(0, "/opt/trn_rl_repo")

import concourse.bacc as bacc
import concourse.bass as bass
import concourse.mybir as mybir
import concourse.tile as tile
from concourse import bass_utils

F16 = mybir.dt.float16
F8 = mybir.dt.float8e4
F32 = mybir.dt.float32
I16 = mybir.dt.int16

H = 8            # heads
D = 16           # head dim
TD = H * D       # 128
QD = 256         # q/k/v feature dim
PD = 64          # edge pair feature dim
R = 4            # src rows of the core grid
C = 2            # dst cols of the core grid
P = 128

AF = mybir.ActivationFunctionType
ALU = mybir.AluOpType


def _wrap16(idx: np.ndarray) -> np.ndarray:
    """dma_gather index layout: [128, n/16] with idx i at (i%16 + 16k, i//16)."""
    n = idx.shape[0]
    assert n % 16 == 0
    w = idx.reshape(n // 16, 16).T.astype(np.int16)  # [16, n/16]
    return np.tile(w, (8, 1))  # replicate across the 8 partition groups


def prepare(q, k, v, edges, edge_index, Wq, Wk, Wv, Wb, bb, Wo, bo):
    N = q.shape[0]
    E = edges.shape[0]
    ntiles_row = math.ceil(N / (R * P))          # tiles per src row
    NROW = ntiles_row * P                        # nodes per row (padded)
    NPAD = NROW * R
    DHALF = NPAD // 2                            # dst-half size
    assert DHALF < 32768, "dst half must fit int16"

    src = np.asarray(edge_index[:, 0], dtype=np.int64)
    dst = np.asarray(edge_index[:, 1], dtype=np.int64)
    deg = np.bincount(src, minlength=N)

    # --- greedy node->tile packing balanced by edge count ---
    T = R * ntiles_row
    order = np.argsort(-deg, kind="stable")
    tile_cnt = np.zeros(T, dtype=np.int64)       # nodes in tile
    tile_edges = np.zeros(T, dtype=np.int64)
    node_tile = np.zeros(N, dtype=np.int32)
    node_slot = np.zeros(N, dtype=np.int32)
    import heapq
    heap = [(0, t) for t in range(T)]
    heapq.heapify(heap)
    for n in order:
        while True:
            e_cnt, t = heapq.heappop(heap)
            if tile_cnt[t] < P:
                break
        node_tile[n] = t
        node_slot[n] = tile_cnt[t]
        tile_cnt[t] += 1
        tile_edges[t] += deg[n]
        if tile_cnt[t] < P:
            heapq.heappush(heap, (tile_edges[t], t))

    row_of_edge = node_tile[src] // ntiles_row
    j_of_edge = (dst // DHALF).astype(np.int64)
    tloc_of_edge = (node_tile[src] % ntiles_row).astype(np.int64)

    # per (core, tile_local) edge counts -> shared block counts per tile slot
    core_of_edge = row_of_edge * C + j_of_edge
    cnt = np.zeros((R * C, ntiles_row), dtype=np.int64)
    np.add.at(cnt, (core_of_edge, tloc_of_edge), 1)
    bpt = np.maximum(1, np.ceil(cnt.max(axis=0) / P).astype(np.int64))  # [ntiles_row]
    blk_off = np.concatenate([[0], np.cumsum(bpt)])   # block offset per tile
    NBLK = int(blk_off[-1])
    ECAP = NBLK * P

    # --- per-core edge arrays ---
    cores = []
    eT_all = np.asarray(edges, dtype=np.float32).T    # [PD, E]
    import ml_dtypes
    F8NP = ml_dtypes.float8_e4m3
    for core in range(R * C):
        i, j = core // C, core % C
        mask = core_of_edge == core
        es, ed, et = src[mask], dst[mask], tloc_of_edge[mask]
        # order edges by tile slot
        ordr = np.argsort(et, kind="stable")
        es, ed, et = es[ordr], ed[ordr], et[ordr]
        # positions: per tile, fill from blk_off[t]*P
        pos = np.zeros(len(es), dtype=np.int64)
        start = 0
        for t in range(ntiles_row):
            c = int((et == t).sum())
            pos[start:start + c] = blk_off[t] * P + np.arange(c)
            start += c
        eidx = np.nonzero(mask)[0][ordr]

        dst_local = np.zeros(ECAP, dtype=np.int16)
        src_rel = np.full(ECAP, 255, dtype=np.int64)
        edgesT = np.zeros((PD + 1, ECAP), dtype=np.float16)
        edgesT[PD, :] = 1.0
        dst_local[pos] = (ed - j * DHALF).astype(np.int16)
        src_rel[pos] = node_slot[es]
        edgesT[:PD, pos] = eT_all[:, eidx].astype(np.float16)
        # one-hot selection matrices (fp8, exact 0/1)
        S_en = np.zeros((ECAP, P), dtype=F8NP)
        valid = src_rel < P
        S_en[np.nonzero(valid)[0], src_rel[valid]] = 1.0
        S_en3 = S_en.reshape(NBLK, P, P)                       # [b, e, n]
        S_mat = np.ascontiguousarray(S_en3.transpose(1, 0, 2)).reshape(P, ECAP)   # [e_part, (b n)]
        ST_mat = np.ascontiguousarray(S_en3.transpose(2, 0, 1)).reshape(P, ECAP)  # [n_part, (b e)]

        # constants: this core projects the FULL khv table of its half j
        qlo = j * DHALF
        qhi = min(qlo + DHALF, N)
        kT = np.zeros((QD, DHALF), dtype=np.float16)
        vT = np.zeros((QD, DHALF), dtype=np.float16)
        kT[:, :qhi - qlo] = np.asarray(k[qlo:qhi], np.float32).T.astype(np.float16)
        vT[:, :qhi - qlo] = np.asarray(v[qlo:qhi], np.float32).T.astype(np.float16)
        # q rows permuted into (tile_local, slot) order for this row i
        qT = np.zeros((QD, NROW), dtype=np.float16)
        rmask = node_tile // ntiles_row == i
        rn = np.nonzero(rmask)[0]
        qpos = (node_tile[rn] % ntiles_row) * P + node_slot[rn]
        qT[:, qpos] = np.asarray(q[rn], np.float32).T.astype(np.float16)

        cores.append(dict(
            dst_idx=_wrap16(dst_local), S_mat=S_mat, ST_mat=ST_mat,
            edgesT=edgesT, kT=kT, vT=vT, qT=qT,
        ))

    norm = D ** -0.5
    consts = dict(
        WkT=np.asarray(Wk, np.float32).T.astype(np.float16),
        WvT=np.asarray(Wv, np.float32).T.astype(np.float16),
        WqT=(np.asarray(Wq, np.float32) * norm).T.astype(np.float16),
        WbT_aug=np.concatenate(
            [np.asarray(Wb, np.float32).T,
             np.asarray(bb, np.float32)[None, :]], axis=0).astype(np.float16),
        WoT=np.asarray(Wo, np.float32).T.astype(np.float16),
        bo_row=np.asarray(bo, np.float32)[None, :].astype(np.float16),
        identity=np.eye(P, dtype=np.float16),
        ones_col=np.ones((1, P), dtype=np.float16),
    )
    meta = dict(N=N, NPAD=NPAD, NROW=NROW, DHALF=DHALF, ntiles_row=ntiles_row,
                NBLK=NBLK, ECAP=ECAP, bpt=bpt.tolist(), blk_off=blk_off.tolist(),
                node_tile=node_tile, node_slot=node_slot, deg=deg)
    return cores, consts, meta


def build_program(meta, gather_batch=3):
    """Build the SPMD bass program. Returns compiled nc."""
    ntr = meta["ntiles_row"]
    NROW, DHALF = meta["NROW"], meta["DHALF"]
    NBLK, ECAP = meta["NBLK"], meta["ECAP"]
    bpt, blk_off = meta["bpt"], meta["blk_off"]
    NKV = DHALF // P      # khv chunks (full half)
    NQ = NROW // P        # qh chunks
    H1 = (ntr + 1) // 2   # tiles in the first reduce/finalize chunk

    nc = bacc.Bacc("TRN2", target_bir_lowering=False, debug=False, num_devices=R * C)
    dt = nc.dram_tensor
    # inputs
    t_dst = dt("dst_idx", [P, ECAP // 16], I16, kind="ExternalInput").ap()
    t_S = dt("S_mat", [P, ECAP], F8, kind="ExternalInput").ap()
    t_ST = dt("ST_mat", [P, ECAP], F8, kind="ExternalInput").ap()
    t_eT = dt("edgesT", [PD + 1, ECAP], F16, kind="ExternalInput").ap()
    t_kT = dt("kT", [QD, DHALF], F16, kind="ExternalInput").ap()
    t_vT = dt("vT", [QD, DHALF], F16, kind="ExternalInput").ap()
    t_qT = dt("qT", [QD, NROW], F16, kind="ExternalInput").ap()
    t_WkT = dt("WkT", [QD, TD], F16, kind="ExternalInput").ap()
    t_WvT = dt("WvT", [QD, TD], F16, kind="ExternalInput").ap()
    t_WqT = dt("WqT", [QD, TD], F16, kind="ExternalInput").ap()
    t_Wb = dt("WbT_aug", [PD + 1, H], F16, kind="ExternalInput").ap()
    t_WoT = dt("WoT", [TD, QD], F16, kind="ExternalInput").ap()
    t_bo = dt("bo_row", [1, QD], F16, kind="ExternalInput").ap()
    t_id = dt("identity", [P, P], F16, kind="ExternalInput").ap()
    t_ones = dt("ones_col", [1, P], F16, kind="ExternalInput").ap()
    # internal DRAM
    t_khv = dt("khv_tab", [DHALF, 2 * TD], F16).ap()
    t_nd_a = dt("nd_part_a", [H1 * P, 136], F16).ap()
    t_ndr_a = dt("nd_red_a", [H1 * P, 136], F16).ap()
    t_nd_b = dt("nd_part_b", [(ntr - H1) * P, 136], F16).ap()
    t_ndr_b = dt("nd_red_b", [(ntr - H1) * P, 136], F16).ap()
    # output (every core finalizes all tiles of its row; host reads j=0 cores)
    t_out = dt("o_out", [ntr * P, QD], F16, kind="ExternalOutput").ap()

    GB = gather_batch

    with tile.TileContext(nc) as tc:
        with (
            tc.tile_pool(name="const", bufs=1) as cpool,
            tc.tile_pool(name="proj", bufs=3) as ppool,
            tc.tile_pool(name="gath", bufs=3) as gpool,
            tc.tile_pool(name="work", bufs=2) as wpool,
            tc.tile_pool(name="out", bufs=2) as opool,
            tc.tile_pool(name="psA", bufs=2, space="PSUM") as psA,
            tc.tile_pool(name="psB", bufs=2, space="PSUM") as psB,
            tc.tile_pool(name="psC", bufs=1, space="PSUM") as psC,
        ):
            # ---- constants to SBUF ----
            c_WkT = cpool.tile([P, 2 * TD], F16)
            nc.sync.dma_start(out=c_WkT[:, 0:TD], in_=t_WkT[0:P, :])
            nc.sync.dma_start(out=c_WkT[:, TD:2 * TD], in_=t_WkT[P:QD, :])
            c_WvT = cpool.tile([P, 2 * TD], F16)
            nc.sync.dma_start(out=c_WvT[:, 0:TD], in_=t_WvT[0:P, :])
            nc.sync.dma_start(out=c_WvT[:, TD:2 * TD], in_=t_WvT[P:QD, :])
            c_WqT = cpool.tile([P, 2 * TD], F16)
            nc.sync.dma_start(out=c_WqT[:, 0:TD], in_=t_WqT[0:P, :])
            nc.sync.dma_start(out=c_WqT[:, TD:2 * TD], in_=t_WqT[P:QD, :])
            c_Wb = cpool.tile([PD + 1, H], F16); nc.sync.dma_start(out=c_Wb[:], in_=t_Wb)
            c_WoT = cpool.tile([TD, QD], F16); nc.sync.dma_start(out=c_WoT[:], in_=t_WoT)
            c_bo = cpool.tile([1, QD], F16); nc.sync.dma_start(out=c_bo[:], in_=t_bo)
            c_id = cpool.tile([P, P], F16); nc.sync.dma_start(out=c_id[:], in_=t_id)
            c_ones = cpool.tile([1, P], F16); nc.sync.dma_start(out=c_ones[:], in_=t_ones)
            c_dsti = cpool.tile([P, ECAP // 16], I16)
            nc.sync.dma_start(out=c_dsti[:], in_=t_dst)
            qh_sb = cpool.tile([P, NQ * TD], F16)

            # ---- phase A: projections (khv -> HBM table, qh -> SBUF) ----
            def project_khv():
                for g0 in range(0, NKV, 8):
                    g1 = min(g0 + 8, NKV)
                    w = (g1 - g0) * P
                    ka = ppool.tile([P, 1024], F16, tag="ka")
                    kb = ppool.tile([P, 1024], F16, tag="kb")
                    va = ppool.tile([P, 1024], F16, tag="va")
                    vb = ppool.tile([P, 1024], F16, tag="vb")
                    nc.sync.dma_start(out=ka[:, :w], in_=t_kT[0:P, g0 * P:g0 * P + w])
                    nc.sync.dma_start(out=kb[:, :w], in_=t_kT[P:QD, g0 * P:g0 * P + w])
                    nc.sync.dma_start(out=va[:, :w], in_=t_vT[0:P, g0 * P:g0 * P + w])
                    nc.sync.dma_start(out=vb[:, :w], in_=t_vT[P:QD, g0 * P:g0 * P + w])
                    stage = ppool.tile([P, 8 * 2 * TD], F16, tag="kvstage")
                    for cc in range(g0, g1):
                        o = (cc - g0) * P
                        psq = psA.tile([P, 2 * TD], F32, tag="projkv")
                        ps = psq[:, 0:TD]
                        nc.tensor.matmul(out=ps, lhsT=ka[:, o:o + P], rhs=c_WkT[:, 0:TD],
                                         start=True, stop=False)
                        nc.tensor.matmul(out=ps, lhsT=kb[:, o:o + P], rhs=c_WkT[:, TD:2 * TD],
                                         start=False, stop=True)
                        ps2 = psq[:, TD:2 * TD]
                        nc.tensor.matmul(out=ps2, lhsT=va[:, o:o + P], rhs=c_WvT[:, 0:TD],
                                         start=True, stop=False)
                        nc.tensor.matmul(out=ps2, lhsT=vb[:, o:o + P], rhs=c_WvT[:, TD:2 * TD],
                                         start=False, stop=True)
                        # one fused 256-wide PSUM->SBUF copy; alternate engines
                        dst_sl = stage[:, (cc - g0) * 256:(cc - g0) * 256 + 256]
                        if cc % 2 == 0:
                            nc.scalar.activation(out=dst_sl, in_=psq[:], func=AF.Copy)
                        else:
                            nc.vector.tensor_copy(out=dst_sl, in_=psq[:])
                    nc.sync.dma_start(
                        out=t_khv[g0 * P:g1 * P, :].rearrange("(c p) w -> p c w", p=P),
                        in_=stage[:, :(g1 - g0) * 256].rearrange(
                            "p (c w) -> p c w", w=256))

            def project_qh():
                for g0 in range(0, NQ, 8):
                    g1 = min(g0 + 8, NQ)
                    w = (g1 - g0) * P
                    ina = ppool.tile([P, 1024], F16, tag="ina")
                    inb = ppool.tile([P, 1024], F16, tag="inb")
                    nc.sync.dma_start(out=ina[:, :w], in_=t_qT[0:P, g0 * P:g0 * P + w])
                    nc.sync.dma_start(out=inb[:, :w], in_=t_qT[P:QD, g0 * P:g0 * P + w])
                    for cc in range(g0, g1):
                        o = (cc - g0) * P
                        psq = psA.tile([P, 2 * TD], F32, tag="projkv")
                        ps = psq[:, 0:TD]
                        nc.tensor.matmul(out=ps, lhsT=ina[:, o:o + P], rhs=c_WqT[:, 0:TD],
                                         start=True, stop=False)
                        nc.tensor.matmul(out=ps, lhsT=inb[:, o:o + P], rhs=c_WqT[:, TD:2 * TD],
                                         start=False, stop=True)
                        if cc % 2 == 0:
                            nc.scalar.activation(out=qh_sb[:, cc * TD:(cc + 1) * TD],
                                                 in_=ps, func=AF.Copy)
                        else:
                            nc.vector.tensor_copy(out=qh_sb[:, cc * TD:(cc + 1) * TD],
                                                  in_=ps)

            project_khv()
            project_qh()

            def reduce_chunk(t_in, t_out):
                nc.gpsimd.collective_compute(
                    "AllReduce", ALU.add,
                    replica_groups=[[0, 1], [2, 3], [4, 5], [6, 7]],
                    ins=[t_in], outs=[t_out])

            # ---- finalize: normalize + output projection, 4 tiles per group ----
            def finalize_group(t0g, t1g):
                ng = t1g - t0g
                src_nd = t_ndr_a if t0g < H1 else t_ndr_b
                tt = t0g if t0g < H1 else t0g - H1
                ndl = opool.tile([P, 4, 136], F16, tag="ndl")
                nc.sync.dma_start(
                    out=ndl[:, :ng, :],
                    in_=src_nd[tt * P:(tt + ng) * P, :].rearrange(
                        "(g p) w -> p g w", p=P))
                rden = opool.tile([P, 4, H], F32, tag="rden")
                # +eps so empty node slots yield 0 instead of 0*inf=NaN
                nc.vector.tensor_scalar_add(out=rden[:, :ng, :],
                                            in0=ndl[:, :ng, TD:TD + H],
                                            scalar1=1e-30)
                nc.vector.reciprocal(out=rden[:, :ng, :], in_=rden[:, :ng, :])
                o_sb = opool.tile([P, 4, TD], F16, tag="o_sb")
                nc.vector.tensor_tensor(
                    out=o_sb[:, :ng, :].rearrange("p g (h d) -> p g h d", h=H),
                    in0=ndl[:, :ng, 0:TD].rearrange("p g (h d) -> p g h d", h=H),
                    in1=rden[:, :ng, :, None].to_broadcast([P, ng, H, D]),
                    op=ALU.mult)
                out_sb = opool.tile([P, 4, QD], F16, tag="out_sb")
                for g in range(ng):
                    ps_oT = psC.tile([P, P], F16, tag="oT")
                    nc.tensor.transpose(out=ps_oT[:], in_=o_sb[:, g, :], identity=c_id[:])
                    oT_sb = opool.tile([P, P], F16, tag="oT_sb")
                    nc.scalar.activation(out=oT_sb[:], in_=ps_oT[:], func=AF.Copy)
                    ps_o = psC.tile([P, QD], F32, tag="ps_o")
                    nc.tensor.matmul(out=ps_o[:], lhsT=oT_sb[:], rhs=c_WoT[:],
                                     start=True, stop=False)
                    nc.tensor.matmul(out=ps_o[:], lhsT=c_ones[:], rhs=c_bo[:],
                                     start=False, stop=True)
                    if g % 2 == 0:
                        nc.scalar.activation(out=out_sb[:, g, :], in_=ps_o[:], func=AF.Copy)
                    else:
                        nc.vector.tensor_copy(out=out_sb[:, g, :], in_=ps_o[:])
                nc.sync.dma_start(
                    out=t_out[t0g * P:(t0g + ng) * P, :].rearrange(
                        "(g p) w -> p g w", p=P),
                    in_=out_sb[:, :ng, :])

            # ---- phase M: main loop over node tiles ----
            batches = []
            t0 = 0
            while t0 < ntr:
                t1 = min(t0 + GB, ntr)
                batches.append((t0, t1))
                t0 = t1

            def compute_tile(t, e0, khv_g, eT, S_sb, ST_sb):
                nb = bpt[t]
                go = blk_off[t] * P - e0     # edge offset in gather batch
                gb = go // P                 # block offset in gather batch
                # qh rows via one-hot matmul; qk product reads PSUM directly
                prod = wpool.tile([P, nb, H, D], F16, tag="prod")
                for b in range(nb):
                    psq2 = psA.tile([P, 2 * TD], F32, tag="projkv")
                    ps_q = psq2[:, 0:TD]
                    nc.tensor.matmul(out=ps_q,
                                     lhsT=ST_sb[:, go + b * P:go + (b + 1) * P],
                                     rhs=qh_sb[:, t * TD:(t + 1) * TD],
                                     start=True, stop=True)
                    nc.vector.tensor_tensor(
                        out=prod[:, b, :, :],
                        in0=ps_q.rearrange("p (h d) -> p h d", h=H),
                        in1=khv_g[:, gb + b, 0:TD].rearrange("p (h d) -> p h d", h=H),
                        op=ALU.mult)
                qk = wpool.tile([P, nb, H], F16, tag="qk")
                with nc.allow_low_precision(reason="f16 qk logits are within tolerance"):
                    nc.vector.reduce_sum(out=qk[:], in_=prod[:], axis=mybir.AxisListType.X)
                # edge-feature logits on PE: eb[e, h] per block
                ps_eb = psB.tile([P, nb * H], F32, tag="eb")
                for b in range(nb):
                    nc.tensor.matmul(out=ps_eb[:, b * H:(b + 1) * H],
                                     lhsT=eT[:, go + b * P:go + (b + 1) * P],
                                     rhs=c_Wb[:], start=True, stop=True)
                attn = wpool.tile([P, nb * H], F32, tag="attn")
                nc.vector.tensor_tensor(out=attn[:], in0=qk[:].rearrange("p b h -> p (b h)"),
                                        in1=ps_eb[:], op=ALU.add)
                w_t = wpool.tile([P, nb, H], F16, tag="w")
                nc.scalar.activation(out=w_t[:].rearrange("p b h -> p (b h)"),
                                     in_=attn[:], func=AF.Exp)
                # rhs = [w*vh | w]
                wv = wpool.tile([P, nb, 136], F16, tag="wv")
                nc.vector.tensor_tensor(
                    out=wv[:, :, 0:TD].rearrange("p b (h d) -> p b h d", h=H),
                    in0=khv_g[:, gb:gb + nb, TD:2 * TD].rearrange("p b (h d) -> p b h d", h=H),
                    in1=w_t[:, :, :, None].to_broadcast([P, nb, H, D]),
                    op=ALU.mult)
                nc.vector.tensor_copy(out=wv[:, :, TD:TD + H], in_=w_t[:])
                # scatter-accumulate into num|den psum
                ps_nd = psB.tile([P, 136], F32, tag="nd")
                for b in range(nb):
                    nc.tensor.matmul(out=ps_nd[:],
                                     lhsT=S_sb[:, go + b * P:go + (b + 1) * P],
                                     rhs=wv[:, b, :],
                                     start=(b == 0), stop=(b == nb - 1))
                nd_sb = opool.tile([P, 136], F16, tag="nd_sb")
                nc.scalar.activation(out=nd_sb[:], in_=ps_nd[:], func=AF.Copy)
                if t < H1:
                    nc.sync.dma_start(out=t_nd_a[t * P:(t + 1) * P, :], in_=nd_sb[:])
                else:
                    nc.sync.dma_start(
                        out=t_nd_b[(t - H1) * P:(t - H1 + 1) * P, :], in_=nd_sb[:])

            # index of the batch holding the final gather: after emitting its
            # gather we can emit chunk-A's reduce without stalling the chain.
            last_bi = len(batches) - 1
            for bi, (b0, b1) in enumerate(batches):
                e0, e1 = blk_off[b0] * P, blk_off[b1] * P
                ne = e1 - e0
                khv_g = gpool.tile([P, ne // P, 2 * TD], F16, tag="khv_g")
                nc.gpsimd.dma_gather(
                    out_ap=khv_g[:], in_ap=t_khv,
                    idxs_ap=c_dsti[:, e0 // 16:e1 // 16],
                    num_idxs=ne, num_idxs_reg=ne, elem_size=2 * TD,
                    single_packet=False)
                if bi == last_bi:
                    # all gathers are in the gpsimd queue; the collective can
                    # follow them without blocking any gather.
                    reduce_chunk(t_nd_a, t_ndr_a)
                eT = gpool.tile([PD + 1, ne], F16, tag="eT")
                nc.sync.dma_start(out=eT[:], in_=t_eT[:, e0:e1])
                S_sb = gpool.tile([P, ne], F8, tag="S_sb")
                nc.sync.dma_start(out=S_sb[:], in_=t_S[:, e0:e1])
                ST_sb = gpool.tile([P, ne], F8, tag="ST_sb")
                nc.sync.dma_start(out=ST_sb[:], in_=t_ST[:, e0:e1])
                for t in range(b0, b1):
                    compute_tile(t, e0, khv_g, eT, S_sb, ST_sb)

            for tf in range(0, H1, 4):
                finalize_group(tf, min(tf + 4, H1))
            reduce_chunk(t_nd_b, t_ndr_b)
            for tf in range(H1, ntr, 4):
                finalize_group(tf, min(tf + 4, ntr))

    nc.compile()
    return nc


_CACHE = {}
LAST_RUN = {}


def kernel(**inputs) -> np.ndarray:
    q = np.asarray(inputs["q"], np.float32)
    k = np.asarray(inputs["k"], np.float32)
    v = np.asarray(inputs["v"], np.float32)
    edges = np.asarray(inputs["edges"], np.float32)
    edge_index = np.asarray(inputs["edge_index"])
    Wq, Wk, Wv = inputs["Wq"], inputs["Wk"], inputs["Wv"]
    Wb, bb, Wo, bo = inputs["Wb"], inputs["bb"], inputs["Wo"], inputs["bo"]

    cores, consts, meta = prepare(q, k, v, edges, edge_index, Wq, Wk, Wv, Wb, bb, Wo, bo)
    N = meta["N"]
    ntr = meta["ntiles_row"]

    key = (q.shape, edges.shape, meta["NBLK"])
    if key not in _CACHE:
        _CACHE[key] = build_program(meta)
    nc = _CACHE[key]

    in_maps = []
    for core in range(R * C):
        m = dict(cores[core])
        m.update({kk: np.ascontiguousarray(vv) for kk, vv in consts.items()})
        in_maps.append({kk: np.ascontiguousarray(vv) for kk, vv in m.items()})

    import os
    if os.environ.get("KERNEL_SIM"):
        from concourse.bass_interp import MultiCoreSim
        sim = MultiCoreSim(nc, num_cores=R * C)
        for ci, core_sim in sim.cores.items():
            for name, arr in in_maps[ci].items():
                core_sim.tensor(name)[:] = arr
        sim.simulate(check_with_hw=False)
        results = [{"o_out": np.array(sim.cores[ci].tensor("o_out"))}
                   for ci in range(R * C)]
    else:
        trace = bool(os.environ.get("KERNEL_TRACE"))
        res = bass_utils.run_bass_kernel_spmd(nc, in_maps, core_ids=list(range(R * C)),
                                              trace=trace)
        LAST_RUN["res"] = res
        results = res.results

    # assemble: core (i, j=0) output has all ntr tiles of row i (both halves
    # reduced identically); use j=0 cores.
    out = np.zeros((meta["NPAD"], QD), np.float32)
    node_tile, node_slot = meta["node_tile"], meta["node_slot"]
    for i in range(R):
        o = results[i * C]["o_out"]  # [ntr*P, QD] f16
        out[i * ntr * P:(i + 1) * ntr * P] = np.asarray(o, np.float32)
    # map back to node ids
    full = np.zeros((N, QD), np.float32)
    rowpos = node_tile * P + node_slot
    full[:, :] = out[rowpos[np.arange(N)]]
    # zero-degree nodes: reference yields bo
    zd = meta["deg"] == 0
    if zd.any():
        full[zd] = np.asarray(bo, np.float32)[None, :]
    return full
